# revision 3
# baseline (speedup 1.0000x reference)
"""Causal multi-head attention block (qkv proj + causal softmax attention + o proj)
for Trainium2, sharded over 8 NeuronCores: data-parallel on batch (B=2),
tensor-parallel on heads (4 heads/core). Each core computes a full o-projection
PARTIAL (its 256 y-features x full o_w slice); the 4 partials per batch are
summed on the host during unsharding -- no on-device collective, so cores run
fully independently (no launch-skew barrier, no CC-engine serialization).

Per-core layout/schedule:
  - x^T [E, T] resident in SBUF; q^T/k^T produced feature-major per 512-token
    chunk; V token-major with a ones column per head (PV then yields y^T and
    the softmax denominator row together)
  - S^T computed in [k, q] layout per (head-pair, 128-k-chunk): both heads'
    scores go into one [128, 1024] PSUM tile (head1 based at col 512) so a
    single ScalarE exp covers both; no max-subtraction (scores bounded ~[-3,3])
  - causal diagonal masking via a [128,128] triangle mask multiply on Pool
  - normalization: denominator row -> ScalarE copy -> DVE reciprocal ->
    Pool partition_broadcast -> DVE multiply (no PE involvement)
  - weave: proj of chunk c+1 interleaves attention block c; o-proj groups of
    blocks 0..2 interleave attention block 3 (they are ScalarE-free PE work,
    placed where exp pressure peaks); o-proj block 3 drains at the tail
"""

import numpy as np
import ml_dtypes

import sys
for _p in ("/opt/trn_rl_repo", "/root/.axon_site/_ro/trn_rl_repo"):
    if _p not in sys.path:
        sys.path.append(_p)

B = 2
T = 2048
E = 1024
H = 16
HD = 64
NCORES = 8
TP = 4               # tensor-parallel degree (cores per batch)
HPC = H // TP        # heads per core = 4
FPC = HPC * HD       # q/k/v feature dims per core = 256
VA = HPC * (HD + 1)  # v features with ones column = 260
QB = 512             # q block / token chunk size
KC = 128             # k chunk

_CACHE = {}


def _build_program(t=T):
    import concourse.bass as bass
    import concourse.bacc as bacc_mod
    import concourse.tile as tile
    import concourse.mybir as mybir

    dt = mybir.dt
    f32 = dt.float32
    bf16 = dt.bfloat16
    AF = mybir.ActivationFunctionType

    nqb = t // QB            # q blocks / token chunks = 4
    nkt = t // KC            # k chunks total = 16

    nc = bacc_mod.Bacc(None, num_devices=NCORES)

    xT = nc.declare_dram_parameter("xT", [E, t], bf16, isOutput=False)
    # weights arrive pre-tiled on the host: partition-row-block k of the
    # logical matrix sits at free-dim block k, so each loads as ONE fat DMA
    wqkT = nc.declare_dram_parameter("wqkT", [128, 8 * 2 * FPC], bf16, isOutput=False)
    bqk = nc.declare_dram_parameter("bqk", [128, 4], f32, isOutput=False)
    wvT = nc.declare_dram_parameter("wvT", [128, 8 * VA], bf16, isOutput=False)
    bva = nc.declare_dram_parameter("bva", [128, VA], f32, isOutput=False)
    owT = nc.declare_dram_parameter("owT", [128, 2 * E], bf16, isOutput=False)
    maskp = nc.declare_dram_parameter("maskp", [128, 128], bf16, isOutput=False)
    ones64 = nc.declare_dram_parameter("ones64", [1, 64], bf16, isOutput=False)
    o_out = nc.declare_dram_parameter("o", [t, E], bf16, isOutput=True)

    with tile.TileContext(nc) as tc, nc.allow_low_precision(
        reason="bf16 compute; tolerance 2e-2"
    ):
        with (
            tc.tile_pool(name="consts", bufs=1) as consts,
            tc.tile_pool(name="res", bufs=1) as res,
            tc.tile_pool(name="pt", bufs=8) as pt_pool,
            tc.tile_pool(name="yu", bufs=4) as yu_pool,
            tc.tile_pool(name="rbs", bufs=4) as rbs_pool,
            tc.tile_pool(name="osb", bufs=4) as osb_pool,
            tc.tile_pool(name="pst", bufs=2, space="PSUM") as pst_pool,
            tc.tile_pool(name="yps", bufs=1, space="PSUM") as yps_pool,
            tc.tile_pool(name="pp", bufs=2, space="PSUM") as pp_pool,
        ):
            # ---- resident tiles -------------------------------------------
            # weights land as two/one fat tiles (one DMA instruction each;
            # DMA issue cadence, not packet size, limits input bandwidth)
            wqk_h = [
                consts.tile([128, 4 * 2 * FPC], bf16, name=f"wqkh{h}", tag=f"wqkh{h}")
                for h in range(2)
            ]

            def wqk_sl(k, ft):
                return wqk_h[k // 4][
                    :, (k % 4) * 2 * FPC + ft * 128:(k % 4) * 2 * FPC + ft * 128 + 128
                ]

            wv_one = consts.tile([128, 8 * VA], bf16, name="wvone", tag="wvone")
            bqk_t = consts.tile([128, 2 * FPC // 128], f32, name="bqkt", tag="bqkt")
            bva_sb = consts.tile([128, VA], f32, name="bva", tag="bva")
            ow_one = consts.tile([128, 2 * E], bf16, name="owone", tag="owone")
            mask_sb = consts.tile([128, 128], bf16, name="mask", tag="mask")
            ones_sb = consts.tile([1, 64], bf16, name="ones", tag="ones")

            # x^T: chunk 0 as single-chunk tiles (early compute), the
            # remaining three chunks as one fat tile per k-row
            x0_sb = [
                res.tile([128, QB], bf16, name=f"x0_{k}", tag=f"x0_{k}")
                for k in range(E // 128)
            ]
            xr_sb = [
                res.tile([128, 3 * QB], bf16, name=f"xr_{k}", tag=f"xr_{k}")
                for k in range(E // 128)
            ]

            def x_sl(k, c):
                if c == 0:
                    return x0_sb[k][:]
                return xr_sb[k][:, (c - 1) * QB:c * QB]
            # q/k feature-major: ft 0..1 = q heads (0,1),(2,3); 2..3 = k
            qk_sb = [
                [
                    res.tile([128, QB], bf16, name=f"qk{ft}_{c}", tag=f"qk{ft}_{c}")
                    for c in range(nqb)
                ]
                for ft in range(2 * FPC // 128)
            ]
            # V token-major (tokens 128i..128i+127) with ones cols
            va_sb = [
                res.tile([128, VA], bf16, name=f"va{i}", tag=f"va{i}")
                for i in range(nkt)
            ]
            # normalized y^T feature-major per (feature-pair dc, chunk)
            y_sb = [
                [
                    res.tile([128, QB], bf16, name=f"y{dc}_{c}", tag=f"y{dc}_{c}")
                    for c in range(nqb)
                ]
                for dc in range(FPC // 128)
            ]
            y_ps = [
                yps_pool.tile([HD + 1, QB], f32, name=f"yps{hh}", tag=f"yps{hh}")
                for hh in range(2)
            ]

            # ---- input DMAs: single Sync queue, priority order ------------
            nc.sync.dma_start(wqk_h[0][:], wqkT[:, 0:4 * 2 * FPC])
            for k in range(4):
                nc.sync.dma_start(x0_sb[k][:], xT[k * 128:(k + 1) * 128, 0:QB])
            nc.sync.dma_start(wqk_h[1][:], wqkT[:, 4 * 2 * FPC:8 * 2 * FPC])
            for k in range(4, E // 128):
                nc.sync.dma_start(x0_sb[k][:], xT[k * 128:(k + 1) * 128, 0:QB])
            nc.sync.dma_start(bqk_t[:], bqk[:])
            nc.sync.dma_start(wv_one[:], wvT[:])
            nc.sync.dma_start(bva_sb[:], bva[:])
            nc.sync.dma_start(mask_sb[:], maskp[:])
            nc.sync.dma_start(ones_sb[:], ones64[:])
            for k in range(E // 128):
                nc.sync.dma_start(
                    xr_sb[k][:], xT[k * 128:(k + 1) * 128, QB:4 * QB]
                )
            nc.sync.dma_start(ow_one[:], owT[:])

            # ---- building blocks ------------------------------------------
            def proj_qk_group(c, ft):
                pp = pp_pool.tile([128, QB], f32, name="pp", tag="pp")
                for k in range(E // 128):
                    nc.tensor.matmul(
                        pp[:],
                        lhsT=wqk_sl(k, ft),
                        rhs=x_sl(k, c),
                        start=(k == 0),
                        stop=(k == E // 128 - 1),
                    )
                nc.scalar.activation(
                    qk_sb[ft][c][:], pp[:], AF.Identity, bias=bqk_t[:, ft:ft + 1]
                )

            def proj_v_group(c, ts):
                pp = pp_pool.tile([128, VA], f32, name="ppv", tag="pp")
                for k in range(E // 128):
                    nc.tensor.matmul(
                        pp[:],
                        lhsT=x_sl(k, c)[:, ts * 128:(ts + 1) * 128],
                        rhs=wv_one[:, k * VA:(k + 1) * VA],
                        start=(k == 0),
                        stop=(k == E // 128 - 1),
                    )
                nc.vector.tensor_add(
                    va_sb[c * (QB // 128) + ts][:], pp[:], bva_sb[:]
                )

            def o_group(tt, ec, tail=False):
                po = pp_pool.tile([128, QB], f32, name="po", tag="pp")
                for dc in range(FPC // 128):
                    nc.tensor.matmul(
                        po[:],
                        lhsT=y_sb[dc][tt // (QB // 128)][
                            :, (tt % (QB // 128)) * 128:(tt % (QB // 128) + 1) * 128
                        ],
                        rhs=ow_one[:, dc * E + ec * 512:dc * E + (ec + 1) * 512],
                        start=(dc == 0),
                        stop=(dc == FPC // 128 - 1),
                    )
                osb = osb_pool.tile([128, 512], bf16, name="osb", tag="osb")
                if tail:
                    nc.scalar.activation(osb[:], po[:], AF.Identity)
                else:
                    nc.vector.tensor_copy(osb[:], po[:])
                nc.sync.dma_start(
                    o_out[tt * 128:(tt + 1) * 128, ec * 512:(ec + 1) * 512], osb[:]
                )

            # ---- attention ------------------------------------------------
            def s_step(qb, j, kc):
                """S matmuls for both heads of pair j vs k-chunk kc, one exp,
                mask if diagonal. Returns (pt tile, W)."""
                dj = kc - qb * (QB // KC)
                off = max(0, dj) * KC
                W = QB - off
                pst = pst_pool.tile([128, 2 * QB], f32, name="pst", tag="pst")
                kt = qk_sb[2 + j][kc // (QB // KC)]
                qt = qk_sb[j][qb]
                kcol = (kc % (QB // KC)) * KC
                for hh in range(2):
                    nc.tensor.matmul(
                        pst[:, hh * QB:hh * QB + W],
                        lhsT=kt[hh * 64:hh * 64 + 64, kcol:kcol + KC],
                        rhs=qt[hh * 64:hh * 64 + 64, off:QB],
                        start=True,
                        stop=True,
                    )
                pt = pt_pool.tile([128, 2 * QB], bf16, name="pt", tag="pt")
                # one exp covers both heads; for diagonal steps the stretch
                # [W:QB) holds stale psum junk whose exp lands in pt columns
                # the PV matmuls never read
                nc.scalar.activation(pt[:, 0:QB + W], pst[:, 0:QB + W], AF.Exp)
                if dj >= 0:
                    # diagonal: zero the strictly-lower triangle of the first
                    # 128 q-columns of each head's region
                    nc.gpsimd.tensor_mul(pt[:, 0:KC], pt[:, 0:KC], mask_sb[:])
                    nc.gpsimd.tensor_mul(
                        pt[:, QB:QB + KC], pt[:, QB:QB + KC], mask_sb[:]
                    )
                return pt, off

            def pv_step(qb, j, kc, pt, off, nkc):
                for hh in range(2):
                    h = 2 * j + hh
                    nc.tensor.matmul(
                        y_ps[hh][:, off:QB],
                        lhsT=va_sb[kc][:, h * (HD + 1):(h + 1) * (HD + 1)],
                        rhs=pt[:, hh * QB:hh * QB + (QB - off)],
                        start=(kc == 0),
                        stop=(kc == nkc - 1),
                    )

            def release(qb, j):
                """Two quick DVE copies per head free the y_ps PSUM banks;
                the rest of the normalization runs off SBUF later."""
                yus = []
                lrows = []
                for hh in range(2):
                    lrow = yu_pool.tile([1, QB], bf16, name="lrow", tag=f"lr{hh}")
                    nc.vector.tensor_copy(lrow[:], y_ps[hh][HD:HD + 1, :])
                    lrows.append(lrow)
                for hh in range(2):
                    yu = yu_pool.tile([HD, QB], bf16, name="yu", tag=f"yu{hh}")
                    nc.vector.tensor_copy(yu[:], y_ps[hh][0:HD, :])
                    yus.append((yu, lrows[hh]))
                return yus

            def norm_rest(qb, j, yus, hhs=(0, 1)):
                for hh in hhs:
                    yu, lrow = yus[hh]
                    rb = pp_pool.tile([64, QB], f32, name="rb", tag="pp")
                    nc.tensor.matmul(
                        rb[:], lhsT=ones_sb[:], rhs=lrow[:],
                        start=True, stop=True,
                    )
                    rbs = rbs_pool.tile([64, QB], f32, name="rbs", tag="rbs")
                    nc.vector.reciprocal_approx_fast(rbs[:], rb[:])
                    nc.vector.tensor_mul(
                        y_sb[j][qb][hh * 64:hh * 64 + 64, :],
                        yu[:],
                        rbs[:],
                    )

            # ---- prologue: projections for chunk 0 ------------------------
            for ft in range(2 * FPC // 128):
                proj_qk_group(0, ft)
            for ts in range(QB // 128):
                proj_v_group(0, ts)

            # ---- woven attention stream -----------------------------------
            def make_fillers(qb):
                fs = []
                if qb < nqb - 1:
                    c = qb + 1
                    for ft in range(2 * FPC // 128):
                        fs.append(lambda c=c, ft=ft: proj_qk_group(c, ft))
                    for ts in range(QB // 128):
                        fs.append(lambda c=c, ts=ts: proj_v_group(c, ts))
                else:
                    for oqb in range(nqb - 1):
                        for ttl in range(QB // 128):
                            for ec in range(E // 512):
                                tt = oqb * (QB // 128) + ttl
                                fs.append(lambda tt=tt, ec=ec: o_group(tt, ec))
                return fs

            pending_norm = []
            # flat step list; the PV of each step trails TWO S-steps behind
            # and crosses j/qb boundaries, so ScalarE sees a gapless exp
            # stream while a finished head pair's PVs/release drain
            pipe = []  # queued (pt, qb, j, kc, off, nkc, last_of_j)
            PVLAG = 3

            def retire(item):
                pt, pqb, pj, pkc, poff, pnkc, plast = item
                pv_step(pqb, pj, pkc, pt, poff, pnkc)
                if plast:
                    yus = release(pqb, pj)
                    for hh in range(2):
                        pending_norm.append(
                            lambda qb=pqb, j=pj, yus=yus, hh=hh:
                            norm_rest(qb, j, yus, hhs=(hh,))
                        )

            for qb in range(nqb):
                fillers = make_fillers(qb)
                nsteps = 2 * (qb + 1) * (QB // KC)
                nf = len(fillers)
                emitted = 0
                step_i = 0
                nkc = (qb + 1) * (QB // KC)
                for j in range(2):
                    for kc in range(nkc):
                        pt, off = s_step(qb, j, kc)
                        if pending_norm and kc in (3, 5):
                            pending_norm.pop(0)()
                        # weave filler(s) between S and the trailing PV so
                        # ScalarE gets a head start on the exp
                        want = (step_i + 1) * nf // nsteps
                        while emitted < want:
                            fillers[emitted]()
                            emitted += 1
                        pipe.append((pt, qb, j, kc, off, nkc, kc == nkc - 1))
                        if len(pipe) > PVLAG:
                            retire(pipe.pop(0))
                        step_i += 1
            while pipe:
                retire(pipe.pop(0))
            while pending_norm:
                pending_norm.pop(0)()

            # ---- tail: o-projection of the last block ---------------------
            for ttl in range(QB // 128):
                for ec in range(E // 512):
                    o_group((nqb - 1) * (QB // 128) + ttl, ec, tail=True)

    nc.finalize()
    return nc


def _shard_inputs(x, qkv_w, qkv_b, o_w, o_b, t=T):
    """Build the 8 per-core input maps."""
    scale = 1.0 / np.sqrt(HD)
    tri = np.triu(np.ones((128, 128), np.float32))  # keep col >= row
    mask_t = np.ascontiguousarray(tri.astype(ml_dtypes.bfloat16))
    in_maps = []
    for c in range(NCORES):
        b, tp = c // TP, c % TP
        qr = slice(FPC * tp, FPC * (tp + 1))
        kr = slice(E + FPC * tp, E + FPC * (tp + 1))
        vr = slice(2 * E + FPC * tp, 2 * E + FPC * (tp + 1))

        xT_c = np.ascontiguousarray(x[b, :t, :].T.astype(ml_dtypes.bfloat16))

        wqkT_c = np.empty((E, 2 * FPC), ml_dtypes.bfloat16)
        wqkT_c[:, :FPC] = qkv_w[qr, :].T * scale
        wqkT_c[:, FPC:] = qkv_w[kr, :].T
        # pre-tiled: row block k becomes free-dim block k -> [128, 8*512]
        wqkT_c = np.ascontiguousarray(
            wqkT_c.reshape(8, 128, 2 * FPC).transpose(1, 0, 2).reshape(128, -1)
        )
        bqk_c = np.concatenate([qkv_b[qr] * scale, qkv_b[kr]])
        bqk_c = np.ascontiguousarray(
            bqk_c.reshape(4, 128).T, dtype=np.float32
        )

        wvT_c = np.zeros((E, VA), ml_dtypes.bfloat16)
        bva_c = np.zeros((1, VA), np.float32)
        wv = qkv_w[vr, :].T  # [E, 256]
        bv = qkv_b[vr]
        for h in range(HPC):
            wvT_c[:, h * (HD + 1):h * (HD + 1) + HD] = wv[:, h * HD:(h + 1) * HD]
            bva_c[0, h * (HD + 1):h * (HD + 1) + HD] = bv[h * HD:(h + 1) * HD]
            bva_c[0, h * (HD + 1) + HD] = 1.0
        bva_t = np.ascontiguousarray(np.broadcast_to(bva_c, (128, VA)))
        wvT_c = np.ascontiguousarray(
            wvT_c.reshape(8, 128, VA).transpose(1, 0, 2).reshape(128, -1)
        )

        owT_c = np.ascontiguousarray(
            o_w[:, FPC * tp:FPC * (tp + 1)].T.astype(ml_dtypes.bfloat16)
        )
        owT_c = np.ascontiguousarray(
            owT_c.reshape(2, 128, E).transpose(1, 0, 2).reshape(128, -1)
        )

        in_maps.append(
            {
                "xT": xT_c,
                "wqkT": wqkT_c,
                "bqk": bqk_c,
                "wvT": wvT_c,
                "bva": bva_t,
                "owT": owT_c,
                "maskp": mask_t,
                "ones64": np.ones((1, 64), ml_dtypes.bfloat16),
            }
        )
    return in_maps


def _run(in_maps, t=T, trace=False):
    from concourse import bass_utils

    key = ("prog", t)
    if key not in _CACHE:
        _CACHE[key] = _build_program(t)
    nc = _CACHE[key]
    res = bass_utils.run_bass_kernel_spmd(
        nc, in_maps, list(range(NCORES)), trace=trace
    )
    return res


def kernel(x, qkv_w, qkv_b, o_w, o_b):
    x = np.asarray(x, np.float32)
    qkv_w = np.asarray(qkv_w, np.float32)
    qkv_b = np.asarray(qkv_b, np.float32)
    o_w = np.asarray(o_w, np.float32)
    o_b = np.asarray(o_b, np.float32)

    in_maps = _shard_inputs(x, qkv_w, qkv_b, o_w, o_b)
    res = _run(in_maps)
    return assemble(res.results, T, o_b)


def assemble(results, t, o_b):
    """Sum the 4 tensor-parallel o-projection partials per batch and add the
    o-projection bias (cheaper on host than as 32 extra DVE ops per core)."""
    ob = np.asarray(o_b, np.float32).reshape(1, E)
    out = np.empty((B, t, E), np.float32)
    for b in range(B):
        acc = np.zeros((t, E), np.float32)
        for tp in range(TP):
            acc += np.asarray(results[b * TP + tp]["o"]).astype(np.float32)
        out[b] = acc + ob
    return out
